# revision 38
# baseline (speedup 1.0000x reference)
"""Trainium2 Bass kernel for nn_AttnTextClassifier.

Reference math (B=256, T=512, V=50000, E=640, D1=D2=512, C=2):
    tokens   = data * mask                     [B, T]
    embedded = emb_table[tokens] * mask[...,None]
    x  = embedded.reshape(B, T*E)              [B, 327680]
    x1 = relu(x @ W1.T + b1)                   [B, 512]
    x2 = relu(x1 @ W2.T + b2)                  [B, 512]
    out = x2 @ Wp.T + bp                       [B, 2]

Distribution (8 cores): tensor-parallel over the T*E contraction dim.
Core c owns tokens t in [64c, 64c+64) -> 40960 contraction columns. The
embedding gather AND the transpose to K-major happen host-side: each core
receives a pre-gathered x^T shard and a pre-transposed W1 shard, both
quantized to fp8 e4m3 (scaled by 2^10 / 2^13 to clear the e4m3 subnormal
range; the product scale 2^23 is divided out when copying PSUM -> SBUF).
Layer 1 is a stream of DoubleRow fp8 matmuls (x^T stationary, W1 moving,
2x ALU rate) accumulating in PSUM over 160 k-pair chunks, fed by one
ramped DMA block schedule per operand (W1 blocks on the sync HWDGE ring,
x^T blocks on the scalar ring, small first blocks so the PE starts
within ~9us). The per-core partial y1 is AllReduced (fp16) and every
core redundantly computes the tiny layers 2/3 in transposed layout.
"""

import os
import sys
import types

import numpy as np
import ml_dtypes

import concourse.bacc as bacc
import concourse.mybir as mybir
import concourse.tile as tile
from concourse.bass_utils import run_bass_kernel_spmd

B, T, V, E = 256, 512, 50000, 640
D1, D2, C = 512, 512, 2
NCORES = 8
TPC = T // NCORES          # 64 tokens per core
KPC = TPC * E              # 40960 contraction columns per core
NKK = KPC // 256           # 160 k-pair chunks (256 contraction rows each)
SCHED = [2, 3, 4, 6, 8, 10, 12, 12, 12, 13, 13, 13, 13, 13, 10, 8, 5, 3]  # sum 160; ramped at the
# front (start latency) AND tapered at the back (the last block's matmul tail is exposed
# after the final DMA lands, delaying the AllReduce doorbell)
MAXBLK = max(SCHED)
SX = 1024.0                # embedding fp8 scale (2^10)
SW = 8192.0                # W1 fp8 scale (2^13)
DESCALE = 1.0 / (SX * SW)

_prog_cache = {}
LAST_RESULTS = None        # BassKernelResults of the last kernel() call


def _install_ntff_hook():
    """Register the axon NTFF profile hook (image's antenv lacks axon_hooks)."""
    if "antenv.axon_hooks" in sys.modules:
        return
    mod = types.ModuleType("antenv.axon_hooks")
    mod._hook = None
    mod.set_axon_ntff_profile_hook = lambda h: setattr(mod, "_hook", h)
    mod.get_axon_ntff_profile_hook = lambda: mod._hook
    sys.modules["antenv.axon_hooks"] = mod
    import antenv

    antenv.axon_hooks = mod
    try:
        from trn_agent_boot.trn_boot import _ntff_profile_via_ctypes

        hook = _ntff_profile_via_ctypes("/opt/axon/libaxon_pjrt.so")
        if hook is not None:
            mod.set_axon_ntff_profile_hook(hook)
    except Exception:
        pass


def _build_program():
    if "nc" in _prog_cache:
        return _prog_cache["nc"]

    nc = bacc.Bacc("TRN2", num_devices=NCORES)
    f8, f16, f32 = mybir.dt.float8e4, mybir.dt.float16, mybir.dt.float32
    Relu = mybir.ActivationFunctionType.Relu
    Copy = mybir.ActivationFunctionType.Copy
    DR = mybir.MatmulPerfMode.DoubleRow
    groups = [list(range(NCORES))]

    wp_ = [
        nc.declare_dram_parameter(f"w{bi}", [128, sz, 2, D1], f8, isOutput=False)
        for bi, sz in enumerate(SCHED)
    ]
    xp_ = [
        nc.declare_dram_parameter(f"x{bi}", [128, sz, 2, B], f8, isOutput=False)
        for bi, sz in enumerate(SCHED)
    ]
    b1t = nc.declare_dram_parameter("b1t", [128, D1 // 128], f32, isOutput=False)
    w2t = nc.declare_dram_parameter("w2t", [D1, D2], f16, isOutput=False)
    b2c = nc.declare_dram_parameter("b2c", [128, D2 // 128], f32, isOutput=False)
    wpt = nc.declare_dram_parameter("wpt", [D2, C], f16, isOutput=False)
    bpc = nc.declare_dram_parameter("bpc", [C, 1], f32, isOutput=False)
    out = nc.declare_dram_parameter("out", [C, B], f32, isOutput=True)

    partial = nc.dram_tensor("partial", [B, D1], f16)
    y1sum = nc.dram_tensor("y1sum", [B, D1], f16, addr_space="Shared")
    warm_in = nc.dram_tensor("warm_in", [2, 1], f32)
    warm_out = nc.dram_tensor("warm_out", [2, 1], f32, addr_space="Shared")

    with tile.TileContext(nc) as tc:
        with (
            tc.tile_pool(name="cpool", bufs=1) as cpool,
            tc.tile_pool(name="xpool", bufs=4) as xpool,
            tc.tile_pool(name="wpool", bufs=4) as wpool,
            tc.tile_pool(name="psum", bufs=1, space="PSUM") as pp,
        ):
            # ---- layer 1: y1_partial[b, n] = x_c[b, k] @ W1c.T[k, n] ----
            # x^T stationary ([128, 2, 128] per b-chunk), W1 moving
            # ([128, 2, 512]), DoubleRow fp8: each matmul contracts 256 rows.
            # Block 0 is tiny and issues first (both halves on the sync ring)
            # so the PE starts ASAP; later x blocks ride the scalar ring.
            ps1 = [pp.tile([128, D1], f32, tag=f"ps1_{bc}", name=f"ps1_{bc}") for bc in range(2)]
            k0 = 0
            for bi, sz in enumerate(SCHED):
                xb = xpool.tile([128, MAXBLK, 2, B], f8, tag="xt")
                nc.sync.dma_start(out=xb[:, :sz, :, :], in_=xp_[bi][:, :, :, :])
                wb = wpool.tile([128, MAXBLK, 2, D1], f8, tag="w1")
                nc.sync.dma_start(out=wb[:, :sz, :, :], in_=wp_[bi][:, :, :, :])
                if bi == 1:
                    # warm up the ncfw collective path concurrently with
                    # layer 1 (after block 0's DMAs so they issue first)
                    nc.scalar.dma_start(out=warm_in[:, :], in_=bpc[:, :])
                    nc.gpsimd.collective_compute(
                        "AllReduce",
                        mybir.AluOpType.add,
                        replica_groups=groups,
                        ins=[warm_in[:, :]],
                        outs=[warm_out[:, :]],
                    )
                for j in range(sz):
                    kk = k0 + j
                    for bc in range(2):
                        nc.tensor.matmul(
                            ps1[bc][:, :],
                            xb[:, j, :, bc * 128 : (bc + 1) * 128],
                            wb[:, j, :, :],
                            start=(kk == 0),
                            stop=(kk == NKK - 1),
                            perf_mode=DR,
                        )
                k0 += sz

            # ---- constants for the post-AllReduce layers (sync ring, after
            # the stream DMAs so they don't compete with the first blocks) ----
            b1_sb = cpool.tile([128, D1 // 128], f32)
            nc.sync.dma_start(out=b1_sb[:, :], in_=b1t[:, :])
            b2_sb = cpool.tile([128, D2 // 128], f32)
            nc.sync.dma_start(out=b2_sb[:, :], in_=b2c[:, :])
            bp_sb = cpool.tile([C, 1], f32)
            nc.sync.dma_start(out=bp_sb[:, :], in_=bpc[:, :])
            w2t_sb = cpool.tile([128, D1 // 128, D2], f16)
            nc.sync.dma_start(
                out=w2t_sb[:, :, :], in_=w2t[:, :].rearrange("(c p) n -> p c n", p=128)
            )
            wpt_sb = cpool.tile([128, D2 // 128, C], f16)
            nc.sync.dma_start(
                out=wpt_sb[:, :, :], in_=wpt[:, :].rearrange("(c p) n -> p c n", p=128)
            )
            # ---- AllReduce of the layer-1 partial (descaled to fp16) ----
            y1_sb = cpool.tile([128, 2, D1], f16)
            for bc in range(2):
                nc.scalar.activation(
                    out=y1_sb[:, bc, :], in_=ps1[bc][:, :], func=Copy, scale=DESCALE
                )
                nc.sync.dma_start(
                    out=partial[bc * 128 : (bc + 1) * 128, :], in_=y1_sb[:, bc, :]
                )
            nc.gpsimd.collective_compute(
                "AllReduce",
                mybir.AluOpType.add,
                replica_groups=groups,
                ins=[partial[:, :]],
                outs=[y1sum[:, :]],
            )
            # ---- fetch y1sum transposed via the xbar DMA-transpose, then
            # fuse bias+relu (bias is per-partition in transposed space) ----
            x1Traw = cpool.tile([128, D1 // 128, B], f16)
            for cc in range(D1 // 128):
                nc.sync.dma_start(
                    out=x1Traw[:, cc, :],
                    in_=y1sum[:, cc * 128 : (cc + 1) * 128],
                    transpose=True,
                )
            x1T = cpool.tile([128, D1 // 128, B], f16)
            for cc in range(D1 // 128):
                nc.scalar.activation(
                    out=x1T[:, cc, :],
                    in_=x1Traw[:, cc, :],
                    func=Relu,
                    bias=b1_sb[:, cc : cc + 1],
                    scale=1.0,
                )

            # ---- layer 2 (transposed): x2T[d2, b] = relu(W2 @ x1 + b2) ----
            ps2 = [pp.tile([128, 2 * B], f32, tag=f"ps2_{i}", name=f"ps2_{i}") for i in range(2)]
            for mc in range(D2 // 128):
                for kc in range(D1 // 128):
                    nc.tensor.matmul(
                        ps2[mc // 2][:, (mc % 2) * B : (mc % 2 + 1) * B],
                        w2t_sb[:, kc, mc * 128 : (mc + 1) * 128],
                        x1T[:, kc, :],
                        start=(kc == 0),
                        stop=(kc == D1 // 128 - 1),
                    )
            x2T = cpool.tile([128, D2 // 128, B], f16)
            for mc in range(D2 // 128):
                nc.scalar.activation(
                    out=x2T[:, mc, :],
                    in_=ps2[mc // 2][:, (mc % 2) * B : (mc % 2 + 1) * B],
                    func=Relu,
                    bias=b2_sb[:, mc : mc + 1],
                    scale=1.0,
                )

            # ---- layer 3 (transposed): out[c, b] = Wp @ x2 + bp ----
            ps3 = pp.tile([C, B], f32, tag="ps3")
            for kc in range(D2 // 128):
                nc.tensor.matmul(
                    ps3[:, :],
                    wpt_sb[:, kc, :],
                    x2T[:, kc, :],
                    start=(kc == 0),
                    stop=(kc == D2 // 128 - 1),
                )
            logits = cpool.tile([C, B], f32)
            nc.scalar.activation(
                out=logits[:, :],
                in_=ps3[:, :],
                func=mybir.ActivationFunctionType.Identity,
                bias=bp_sb[:, 0:1],
                scale=1.0,
            )
            nc.sync.dma_start(out=out[:, :], in_=logits[:, :])

    nc.finalize()
    _prog_cache["nc"] = nc
    return nc


def _host_prep(data, mask, emb_table, W1, b1, W2, b2, Wp, bp):
    f8 = ml_dtypes.float8_e4m3
    data = np.asarray(data)
    mask = np.asarray(mask)
    tokens = np.where(mask != 0, data, V).astype(np.int64)  # V -> zero row
    embq = np.vstack(
        [
            (np.asarray(emb_table).astype(np.float32) * SX).astype(f8),
            np.zeros((1, E), f8),
        ]
    )
    W1q = (np.asarray(W1).astype(np.float32) * SW).astype(f8)  # [D1, T*E]
    b1_in = np.asarray(b1).astype(np.float32).reshape(D1 // 128, 128).T.copy()
    W2T = np.ascontiguousarray(np.asarray(W2).astype(np.float16).T)
    b2_in = np.asarray(b2).astype(np.float32).reshape(D2 // 128, 128).T.copy()
    WpT = np.ascontiguousarray(np.asarray(Wp).astype(np.float16).T)
    bp_in = np.asarray(bp).astype(np.float32).reshape(C, 1)

    in_maps = []
    for c in range(NCORES):
        toks_c = tokens[:, c * TPC : (c + 1) * TPC]  # [B, TPC]
        # x^T[k, b] and W1^T[k, n] with k = t*E + e, regrouped per DMA block:
        # blk[p, j, i, :] = row k = (k0 + j)*256 + i*128 + p
        xk = embq[toks_c].transpose(1, 2, 0).reshape(NKK, 2, 128, B)
        W1k = np.ascontiguousarray(W1q[:, c * KPC : (c + 1) * KPC].T).reshape(
            NKK, 2, 128, D1
        )
        m = {
            "b1t": b1_in,
            "w2t": W2T,
            "b2c": b2_in,
            "wpt": WpT,
            "bpc": bp_in,
        }
        k0 = 0
        for bi, sz in enumerate(SCHED):
            m[f"w{bi}"] = np.ascontiguousarray(
                W1k[k0 : k0 + sz].transpose(2, 0, 1, 3)
            )
            m[f"x{bi}"] = np.ascontiguousarray(xk[k0 : k0 + sz].transpose(2, 0, 1, 3))
            k0 += sz
        in_maps.append(m)
    return in_maps


def kernel(data, mask, emb_table, W1, b1, W2, b2, Wp, bp):
    global LAST_RESULTS
    nc = _build_program()
    in_maps = _host_prep(data, mask, emb_table, W1, b1, W2, b2, Wp, bp)

    trace = os.environ.get("KERNEL_TRACE", "0") == "1"
    if trace:
        _install_ntff_hook()
    br = run_bass_kernel_spmd(nc, in_maps, list(range(NCORES)), trace=trace)
    LAST_RESULTS = br
    return np.ascontiguousarray(br.results[0]["out"].T.astype(np.float32))
